# revision 1
# baseline (speedup 1.0000x reference)
"""MenuLoss Trainium2 kernel.

Math: the loss needs, per batch b, cal[b] = (1/700)*sum_j amt_bj * p(x_bj) for two
evals (true ids continuous, pred ids rounded), where p is a degree-446 Chebyshev
series.  Fold p into a bilinear form p(x) = sum_{a<22, r<21} G[a,r]*T_a(y)*T_r(x),
y = T_21(x) (exact: 22*21=462 >= 447; G solved on host in float64 from the runtime
coeffs).  On device, build the 43 basis functions per element with Chebyshev
recurrence ladders (even indices via ACT Square: T_2m = 2*T_m^2 - 1; odd via DVE
double-step: T_{r+2} = 2*T_2*T_r - T_{r-2}), fold amt into the T_a(y) side, and
contract over elements with TensorE matmuls accumulating per-batch Grams in PSUM.
A signed G-matrix contraction then yields calT[b]-calP[b] directly; penalties are
fused elementwise maps with accumulated reductions.  8-way batch data parallel,
per-core scalar partials summed on host.
"""
import functools
import sys
import types
import numpy as np

# this container's axon build lacks the NTFF profile hook module; stub it so
# run_bass_kernel_spmd(trace=True) degrades to an untraced run instead of dying
if "antenv.axon_hooks" not in sys.modules:
    _m = types.ModuleType("antenv.axon_hooks")
    _m.get_axon_ntff_profile_hook = lambda: None
    sys.modules["antenv.axon_hooks"] = _m

import concourse.bacc as bacc
import concourse.bass as bass
import concourse.mybir as mybir
import concourse.tile as tile
from concourse.bass_utils import run_bass_kernel_spmd

AFT = mybir.ActivationFunctionType
ALU = mybir.AluOpType
FP32 = mybir.dt.float32

N_CORES = 8
B, J = 512, 7 * 16 * 64          # 512 batches, 7168 elements/batch
BC = B // N_CORES                # 64 batches per core
CHUNKS = J // 128                # 56 contraction chunks per batch
K, A = 21, 22                    # p(x) = sum G[a,r] T_a(T_K(x)) T_r(x)
NB = 2 * K                       # 42 combined U cols (U_T | U_P); W rows 44
SL = 8                           # batches per slice
NSL = BC // SL                   # 8 slices
C = SL * CHUNKS                  # 448 columns per slice
R2 = np.sqrt(2.0).astype(np.float32) if False else float(np.sqrt(2.0))


def _shift(i):
    # device basis stores T_i + 1 for even i >= 4 (saves the -1 pass)
    return 1.0 if (i >= 4 and i % 2 == 0) else 0.0


def _fold_G(coeffs: np.ndarray) -> np.ndarray:
    """Solve G[A,K] s.t. sum G[a,r] (T_a(T_K(x))+s_a)(T_r(x)+s_r) == chebval."""
    NN = A * K
    M = np.zeros((NN, NN), np.float64)
    for a in range(A):
        sa = _shift(a)
        for r in range(K):
            sr = _shift(r)
            row = a * K + r
            M[row, a * K + r] += 0.5
            M[row, abs(a * K - r)] += 0.5
            M[row, a * K] += sr
            M[row, r] += sa * 1.0 if a > 0 else sa  # T_r term
            M[row, 0] += sa * sr
    c = np.zeros(NN, np.float64)
    c[: len(coeffs)] = coeffs
    g = np.linalg.solve(M.T, c)
    return g.reshape(A, K)


def _build_ladder(nc, bias_r2, tmp_pool, UU, ubase, nb, seed_kind, ids=None, y=None):
    """Write T_0..T_{nb-1} into UU[:, ubase+r, :] (basis-major [128, *, C]).

    seed_kind 'x': seeds from ids tile (x = ids/111 - 1); returns y=T_21 tile
    seed_kind 'y': seeds from given y tile.
    Returns the T_K tile for seed_kind 'x' (to seed the second level), else None.
    """
    sl = lambda r: UU[:, :, ubase + r]
    q = tmp_pool.tile([128, C], FP32, tag="lad_q")
    u = tmp_pool.tile([128, C], FP32, tag="lad_u")
    m = tmp_pool.tile([128, C], FP32, tag="lad_m")
    if seed_kind == "x":
        # T1 = ids/111 - 1 ; 2x^2 via ACT Square(sqrt2/111 * ids - sqrt2)
        nc.vector.tensor_scalar(sl(1), ids, 1.0 / 111.0, 1.0, ALU.mult, ALU.subtract)
        nc.scalar.activation(q[:], ids, AFT.Square, scale=R2 / 111.0, bias=bias_r2)
        s2 = tmp_pool.tile([128, C], FP32, tag="lad_s2")
        nc.vector.tensor_scalar(s2[:], ids, 2.0 / 111.0, 2.0, ALU.mult, ALU.subtract)
    else:
        nc.scalar.copy(sl(1), y)
        nc.scalar.activation(q[:], y, AFT.Square, scale=R2, bias=0.0)
        s2 = tmp_pool.tile([128, C], FP32, tag="lad_s2")
        nc.vector.tensor_scalar_mul(s2[:], y, 2.0)
    nc.gpsimd.memset(sl(0), 1.0)
    nc.vector.tensor_scalar_sub(sl(2), q[:], 1.0)          # T2 = 2x^2-1
    nc.vector.tensor_scalar(u[:], q[:], 2.0, 2.0, ALU.mult, ALU.subtract)  # u=2*T2
    # T3 = 2x*T2 - x
    nc.vector.tensor_tensor(m[:], s2[:], sl(2), ALU.mult)
    nc.vector.scalar_tensor_tensor(sl(3), m[:], 1.0, sl(1), ALU.mult, ALU.subtract)
    # T4 (shifted: slot = 2*T2^2 = T4+1)
    nc.scalar.activation(sl(4), sl(2), AFT.Square, scale=R2, bias=0.0)
    # odd chain on DVE: T_r = u*T_{r-2} - T_{r-4}
    for r in range(5, nb, 2):
        nc.vector.tensor_tensor(m[:], u[:], sl(r - 2), ALU.mult)
        nc.vector.scalar_tensor_tensor(sl(r), m[:], 1.0, sl(r - 4), ALU.mult,
                                       ALU.subtract)
    # evens >= 6: shifted squares (inputs with even m>=4 are shifted -> bias)
    for r in range(6, nb, 2):
        m2 = r // 2
        b = bias_r2 if (m2 >= 4 and m2 % 2 == 0) else 0.0
        nc.scalar.activation(sl(r), sl(m2), AFT.Square, scale=R2, bias=b)
    if seed_kind == "x":
        # y = T_21 = u*T_19 - T_17
        yt = tmp_pool.tile([128, C], FP32, tag="lad_y")
        nc.vector.tensor_tensor(m[:], u[:], sl(K - 2), ALU.mult)
        nc.vector.scalar_tensor_tensor(yt[:], m[:], 1.0, sl(K - 4), ALU.mult,
                                       ALU.subtract)
        return yt
    return None


def _build(slices=NSL):
    nc = bacc.Bacc("TRN2", target_bir_lowering=False, debug=False, num_devices=1)
    yp = nc.dram_tensor("yp", [BC, J, 2], FP32, kind="ExternalInput")
    yt = nc.dram_tensor("yt", [BC, J, 2], FP32, kind="ExternalInput")
    # signed/scaled G layout [44, SL*43]: rows 0..21 (+G/700) hit the T-eval
    # block (cols b*43+r, r<21); rows 22..43 (-G/700) hit P-block (cols 21+r).
    gc = nc.dram_tensor("gc", [2 * A, SL * NB], FP32, kind="ExternalInput")
    out3 = nc.dram_tensor("out3", [1, 4], FP32, kind="ExternalOutput")

    bias_np = np.broadcast_to(np.array([-np.sqrt(2.0), -222.0], np.float32),
                              (128, 2)).copy()
    bias_dram = nc.inline_tensor(bias_np, name="bias_const")
    yp_r = yp.ap().rearrange("b (c p) t -> p (b c) t", p=128)
    yt_r = yt.ap().rearrange("b (c p) t -> p (b c) t", p=128)

    with tile.TileContext(nc) as tc:
        with (
            tc.tile_pool(name="data", bufs=2) as data_pool,
            tc.tile_pool(name="basis", bufs=1) as basis_pool,
            tc.tile_pool(name="tmp", bufs=1) as tmp_pool,
            tc.tile_pool(name="small", bufs=1) as small_pool,
            tc.tile_pool(name="psum", bufs=2, space="PSUM") as psum_pool,
            tc.tile_pool(name="psc", bufs=2, space="PSUM") as psc_pool,
        ):
            gct = small_pool.tile([2 * A, SL * NB], FP32)
            nc.sync.dma_start(gct[:], gc.ap())
            bias_t = small_pool.tile([128, 2], FP32)
            nc.sync.dma_start(bias_t[:], bias_dram.ap())
            ones44 = small_pool.tile([2 * A, 1], FP32)
            nc.gpsimd.memset(ones44[:], 1.0)
            ones128 = small_pool.tile([128, 1], FP32)
            nc.gpsimd.memset(ones128[:], 1.0)
            diffs = small_pool.tile([1, BC], FP32)
            pen_parts = small_pool.tile([128, NSL], FP32)
            ir_parts = small_pool.tile([128, NSL], FP32)

            for s in range(slices):
                cs = slice(s * C, (s + 1) * C)
                dP = data_pool.tile([128, C, 2], FP32, tag="dP")
                dT = data_pool.tile([128, C, 2], FP32, tag="dT")
                nc.sync.dma_start(dP[:], yp_r[:, cs, :])
                nc.sync.dma_start(dT[:], yt_r[:, cs, :])
                idsP, amtP = dP[:, :, 0], dP[:, :, 1]
                idsT, amtT = dT[:, :, 0], dT[:, :, 1]

                # round pred ids (cast rte matches jnp.round)
                ki = tmp_pool.tile([128, C], mybir.dt.int32, tag="lad_q")
                kf = tmp_pool.tile([128, C], FP32, tag="kf")
                nc.vector.tensor_copy(ki[:], idsP)
                nc.vector.tensor_copy(kf[:], ki[:])

                # penalties on pred slice (raw ids/amt)
                t_i = tmp_pool.tile([128, C], FP32, tag="lad_u")
                t_a = tmp_pool.tile([128, C], FP32, tag="lad_m")
                pm = tmp_pool.tile([128, C], FP32, tag="lad_s2")
                nc.scalar.activation(t_i[:], idsP, AFT.Tanh, scale=4.0)
                nc.scalar.activation(t_a[:], amtP, AFT.Tanh, scale=4.0)
                nc.gpsimd.tensor_tensor(pm[:], t_i[:], t_a[:], ALU.mult)
                nc.vector.scalar_tensor_tensor(pm[:], pm[:], -2.0, t_i[:],
                                               ALU.mult, ALU.add)
                nc.vector.scalar_tensor_tensor(
                    pm[:], pm[:], 1.0, t_a[:], ALU.mult, ALU.add,
                    accum_out=pen_parts[:, s:s + 1])
                ir_t = tmp_pool.tile([128, C], FP32, tag="lad_y")
                nc.scalar.activation(ir_t[:], idsP, AFT.Relu, bias=bias_t[:, 1:2],
                                     accum_out=ir_parts[:, s:s + 1])

                # basis tiles: UU [128, 43, C] (U_T 0..20 | U_P 21..41+1),
                # WW [128, 44, C] (amt*V: T rows 0..21 | P rows 22..43)
                UU = basis_pool.tile([128, C, NB], FP32, tag="UU")
                WW = basis_pool.tile([128, C, 2 * A], FP32, tag="WW")

                for (ids_ap, amt_ap, ub, wb) in (
                    (idsT, amtT, 0, 0),
                    (kf[:], amtP, K, A),
                ):
                    y_t = _build_ladder(nc, bias_t[:, 0:1], tmp_pool, UU, ub, K,
                                        "x", ids=ids_ap)
                    _build_ladder(nc, bias_t[:, 0:1], tmp_pool, WW, wb, A, "y", y=y_t[:])
                    # fold amt into V rows in place (split DVE / gpsimd)
                    for a in range(A):
                        eng = nc.gpsimd if a < 16 else nc.vector
                        eng.tensor_tensor(WW[:, :, wb + a], WW[:, :, wb + a], amt_ap,
                                          ALU.mult)

                # per-batch Grams: psum [44, SL*43], accumulate over 56 chunks
                ps = psum_pool.tile([2 * A, SL * NB], FP32, tag="gram")
                for b in range(SL):
                    for c in range(CHUNKS):
                        j = b * CHUNKS + c
                        nc.tensor.matmul(
                            ps[:, b * NB:(b + 1) * NB],
                            WW[:, j, :], UU[:, j, :],
                            start=(c == 0), stop=(c == CHUNKS - 1))
                # contract with signed G: diffs[b] = calT - calP
                gs = tmp_pool.tile([2 * A, SL * NB], FP32, tag="gs")
                nc.vector.scalar_tensor_tensor(gs[:], ps[:], 1.0, gct[:],
                                               ALU.mult, ALU.mult)
                ps2 = psc_pool.tile([1, SL * NB], FP32, tag="colsum")
                nc.tensor.matmul(ps2[:], ones44[:], gs[:], start=True, stop=True)
                sall = tmp_pool.tile([1, SL * NB], FP32, tag="sall")
                nc.scalar.copy(sall[:], ps2[:])
                nc.vector.tensor_reduce(
                    diffs[:, s * SL:(s + 1) * SL],
                    sall[:].rearrange("p (b n) -> p b n", n=NB),
                    mybir.AxisListType.X, ALU.add)

            # final: sum_b diffs^2, penalty partition-sums
            dsq = small_pool.tile([1, BC], FP32)
            nc.scalar.activation(dsq[:], diffs[:], AFT.Square)
            v0 = small_pool.tile([1, 1], FP32)
            nc.vector.tensor_reduce(v0[:], dsq[:], mybir.AxisListType.X, ALU.add)
            pen_red = small_pool.tile([128, 2], FP32)
            nc.vector.tensor_reduce(pen_red[:, 0:1], pen_parts[:],
                                    mybir.AxisListType.X, ALU.add)
            nc.vector.tensor_reduce(pen_red[:, 1:2], ir_parts[:],
                                    mybir.AxisListType.X, ALU.add)
            ps3 = psc_pool.tile([1, 2], FP32, tag="pen")
            nc.tensor.matmul(ps3[:], ones128[:], pen_red[:], start=True, stop=True)
            ot = small_pool.tile([1, 4], FP32)
            nc.vector.tensor_copy(ot[:, 0:1], v0[:])
            nc.vector.tensor_copy(ot[:, 1:3], ps3[:])
            nc.gpsimd.memset(ot[:, 3:4], 0.0)
            nc.sync.dma_start(out3.ap(), ot[:])
    nc.compile()
    return nc


@functools.lru_cache(maxsize=2)
def _compiled():
    return _build()


def kernel(y_pred: np.ndarray, y: np.ndarray, calories_coeffs: np.ndarray,
           _trace: bool = False):
    G = _fold_G(np.asarray(calories_coeffs, np.float64))
    gc = np.zeros((2 * A, SL * NB), np.float32)
    for b in range(SL):
        gc[:A, b * NB:b * NB + K] = (G / 700.0).astype(np.float32)
        gc[A:, b * NB + K:b * NB + 2 * K] = (-G / 700.0).astype(np.float32)

    ypf = np.ascontiguousarray(y_pred.reshape(B, J, 2), np.float32)
    ytf = np.ascontiguousarray(y.reshape(B, J, 2), np.float32)
    in_maps = []
    for i in range(N_CORES):
        in_maps.append({
            "yp": ypf[i * BC:(i + 1) * BC],
            "yt": ytf[i * BC:(i + 1) * BC],
            "gc": gc,
        })
    nc = _compiled()
    res = run_bass_kernel_spmd(nc, in_maps, list(range(N_CORES)), trace=_trace)
    parts = np.stack([r["out3"][0] for r in res.results])  # [8, 4]
    tot = parts.sum(axis=0)
    loss = (tot[0] + tot[1] + tot[2]) / float(B)
    out = np.float32(loss)
    if _trace:
        return out, res
    return out



# revision 6
# speedup vs baseline: 3.2923x; 3.2923x over previous
"""MenuLoss Trainium2 kernel (v2: fp16 folded ladders + per-batch Grams, host contract).

Math: loss = mean_b[pen_b] + mean_b[((calT_b - calP_b)/700)^2], where cal_b are
amt-weighted sums of a degree-446 Chebyshev series p over ids.  Device computes,
per batch, a Gram matrix M_b = S^T W (contraction over the batch's 7168 elements
via TensorE, PSUM-accumulated over 56 chunks of 128), where:
  stationary S rows (59): [1, T_1..T_29(x) (evens stored +1), onehot_s(rem) x28,
                           tanh(4*idsP)]          -- x = idsT/111-1
  moving   W rows (25): [amtT*T_f(y) f<15 (y=T_30(x), fp32 cascade via
                           T2(T3(T5)) then clamped), amtP*onehot_q(k//28) x8,
                           tanh(4*amtP), 1]
Basis built with fp16 Chebyshev double-step ladders (odd chain on DVE 2x, even
terms via ACT Square with shifted (+1) storage), one-hots via DVE 4x is_equal.
True side: p(x) = sum G_T[f,r] T_f(T_30(x)) U_r(x) (G_T solved on host, fp64);
pred side exact: ids rounded on ACT (RNE), p(28q+s) table G_P.  Penalties ride
along as extra Gram rows; id_range penalty is identically 0 (ids <= 222).
Host unshards the 8 cores' Grams and contracts in fp64.
"""
import functools
import sys
import types
import numpy as np
import numpy.polynomial.chebyshev as Ch

# this container's axon build lacks the NTFF profile hook module; stub it so
# run_bass_kernel_spmd(trace=True) degrades to an untraced run instead of dying
if "antenv.axon_hooks" not in sys.modules:
    _m = types.ModuleType("antenv.axon_hooks")
    _m.get_axon_ntff_profile_hook = lambda: None
    sys.modules["antenv.axon_hooks"] = _m

import concourse.bacc as bacc
import concourse.mybir as mybir
import concourse.tile as tile
from concourse.bass_utils import run_bass_kernel_spmd

AFT = mybir.ActivationFunctionType
ALU = mybir.AluOpType
FP32 = mybir.dt.float32
FP16 = mybir.dt.float16
I16 = mybir.dt.int16
I32 = mybir.dt.int32

N_CORES = 8
B, J = 512, 7 * 16 * 64          # 512 batches, 7168 elements/batch
BC = B // N_CORES                # 64 batches per core
SL = 8                           # batches per slice
NSL = BC // SL                   # 8 slices
CH = 56                          # chunks (columns) per batch
C = SL * CH                      # 448 columns per slice

UT = 30                          # true stationary basis: U_0..U_29
FT = 15                          # true moving basis: W_0..W_14, y = T_30
QP, SP = 8, 28                   # pred split k = 28q + s
NST = 1 + (UT - 1) + SP + 1      # 59 stationary rows
NMV = FT + QP + 1 + 1            # 25 moving rows
R_OH = 1 + (UT - 1)              # stationary row where onehot_s starts (30)
R_TI = R_OH + SP                 # 58
M_WQ = FT                        # moving row where folded q-onehots start (15)
M_TA = M_WQ + QP                 # 23
M_ONE = M_TA + 1                 # 24

RT2 = float(np.sqrt(2.0))


def _build():
    nc = bacc.Bacc("TRN2", target_bir_lowering=False, debug=False, num_devices=1)
    yp = nc.dram_tensor("yp", [BC, J, 2], FP32, kind="ExternalInput")
    yt = nc.dram_tensor("yt", [BC, J, 2], FP32, kind="ExternalInput")
    gout = nc.dram_tensor("gout", [NST, NSL, SL * NMV], FP32, kind="ExternalOutput")

    bias_np = np.broadcast_to(
        np.array([-RT2, -1.0, -27.0 / 56.0], np.float32), (128, 3)).copy()
    bias_dram = nc.inline_tensor(bias_np, name="bias_const")
    yp_r = yp.ap().rearrange("b (p c) t -> p b c t", p=128)
    yt_r = yt.ap().rearrange("b (p c) t -> p b c t", p=128)

    with tile.TileContext(nc) as tc:
        with (
            tc.tile_pool(name="data", bufs=2) as dpool,
            tc.tile_pool(name="stat", bufs=2) as spool,
            tc.tile_pool(name="mov", bufs=2) as mpool,
            tc.tile_pool(name="scr", bufs=1) as scr,
            tc.tile_pool(name="outp", bufs=2) as opool,
            tc.tile_pool(name="psum", bufs=2, space="PSUM") as ppool,
        ):
            bias_t = scr.tile([128, 3], FP32, tag="bias")
            nc.sync.dma_start(bias_t[:], bias_dram.ap())
            b_rt2 = bias_t[:, 0:1]
            b_m1 = bias_t[:, 1:2]
            b_q = bias_t[:, 2:3]

            for s in range(NSL):
                bs = slice(s * SL, (s + 1) * SL)
                dT = dpool.tile([128, SL, CH, 2], FP32, tag="dT")
                dP = dpool.tile([128, SL, CH, 2], FP32, tag="dP")
                nc.sync.dma_start(dT[:], yt_r[:, bs, :, :])
                nc.sync.dma_start(dP[:], yp_r[:, bs, :, :])
                flat = lambda ap: ap.rearrange("p b c -> p (b c)")
                idsT, amtT = flat(dT[:, :, :, 0]), flat(dT[:, :, :, 1])
                idsP, amtP = flat(dP[:, :, :, 0]), flat(dP[:, :, :, 1])

                ST = spool.tile([128, NST, C], FP16, tag="ST")
                MV = mpool.tile([128, NMV, C], FP16, tag="MV")
                U = lambda r: ST[:, r, :]
                W = lambda f: MV[:, f, :]

                nc.gpsimd.memset(U(0), 1.0)
                nc.gpsimd.memset(MV[:, M_ONE, :], 1.0)

                # ---- true side: U ladder (x = idsT/111 - 1) ----
                nc.scalar.activation(U(1), idsT, AFT.Copy, scale=1.0 / 111.0,
                                     bias=-1.0)
                nc.scalar.activation(U(2), idsT, AFT.Square, scale=RT2 / 111.0,
                                     bias=b_rt2)              # 2x^2 = T2+1
                u = scr.tile([128, C], FP16, tag="u")
                v = scr.tile([128, C], FP16, tag="v")
                nc.vector.tensor_scalar(u[:], U(2), 2.0, 2.0, ALU.mult,
                                        ALU.subtract)          # u = 2*T2
                nc.vector.tensor_scalar(v[:], u[:], 1.0, 1.0, ALU.mult,
                                        ALU.subtract)          # v = 2*T2 - 1
                nc.vector.tensor_tensor(U(3), v[:], U(1), ALU.mult)
                mo = scr.tile([128, C], FP16, tag="mo")
                for r in range(5, UT, 2):
                    nc.vector.tensor_tensor(mo[:], u[:], U(r - 2), ALU.mult)
                    nc.vector.tensor_tensor(U(r), mo[:], U(r - 4), ALU.subtract)
                for r in range(4, UT, 2):
                    h = r // 2
                    if h % 2 == 0:     # input stored shifted (T_h + 1)
                        nc.scalar.activation(U(r), U(h), AFT.Square, scale=RT2,
                                             bias=b_rt2)
                    else:
                        nc.scalar.activation(U(r), U(h), AFT.Square, scale=RT2,
                                             bias=0.0)

                # ---- y = T30(x) via fp32 cascade T2(T3(T5(x))), clamped ----
                x32 = scr.tile([128, C], FP32, tag="x32")
                w32 = scr.tile([128, C], FP32, tag="w32")
                a32 = scr.tile([128, C], FP32, tag="a32")
                t5 = scr.tile([128, C], FP32, tag="t5")
                t15 = scr.tile([128, C], FP32, tag="t15")
                nc.vector.tensor_scalar(x32[:], idsT, 1.0 / 111.0, 1.0,
                                        ALU.mult, ALU.subtract)
                nc.scalar.activation(w32[:], idsT, AFT.Square, scale=1.0 / 111.0,
                                     bias=b_m1)               # x^2
                nc.vector.tensor_scalar(a32[:], w32[:], 16.0, 20.0, ALU.mult,
                                        ALU.subtract)
                nc.vector.tensor_tensor(a32[:], a32[:], w32[:], ALU.mult)
                nc.vector.scalar_tensor_tensor(t5[:], a32[:], 5.0, x32[:],
                                               ALU.add, ALU.mult)   # T5
                nc.scalar.activation(w32[:], t5[:], AFT.Square, scale=RT2,
                                     bias=0.0)                # 2*T5^2
                nc.vector.tensor_scalar(a32[:], w32[:], 2.0, 3.0, ALU.mult,
                                        ALU.subtract)
                nc.vector.tensor_tensor(t15[:], a32[:], t5[:], ALU.mult)  # T15
                ys = scr.tile([128, C], FP16, tag="ys")
                nc.scalar.activation(ys[:], t15[:], AFT.Square, scale=RT2,
                                     bias=0.0)                # 2*T15^2 = T30+1
                ysc = scr.tile([128, C], FP16, tag="ysc")
                nc.vector.tensor_scalar_min(ysc[:], ys[:], 2.0)

                # ---- true moving side: W_f = amtT * T_f(y) ----
                nc.gpsimd.tensor_copy(W(0), amtT)
                mw = scr.tile([128, C], FP16, tag="mw")
                nc.vector.tensor_tensor(mw[:], W(0), ysc[:], ALU.mult)
                nc.vector.tensor_tensor(W(1), mw[:], W(0), ALU.subtract)
                uy = scr.tile([128, C], FP16, tag="uy")
                nc.vector.tensor_scalar(uy[:], ysc[:], 2.0, 2.0, ALU.mult,
                                        ALU.subtract)          # 2y
                for f in range(2, FT):
                    nc.vector.tensor_tensor(mw[:], uy[:], W(f - 1), ALU.mult)
                    nc.vector.tensor_tensor(W(f), mw[:], W(f - 2), ALU.subtract)

                # ---- pred side ----
                k32 = scr.tile([128, C], I32, tag="k32")
                nc.scalar.activation(k32[:], idsP, AFT.Copy, scale=1.0, bias=0.0)
                q16 = scr.tile([128, C], I16, tag="q16")
                nc.scalar.activation(q16[:], k32[:], AFT.Copy, scale=1.0 / 28.0,
                                     bias=-27.0 / 56.0)       # floor(k/28) (RNE)
                rem = scr.tile([128, C], FP16, tag="rem")
                nc.vector.scalar_tensor_tensor(rem[:], q16[:], -28.0, k32[:],
                                               ALU.mult, ALU.add)  # k - 28q
                for sv in range(SP):
                    nc.vector.tensor_scalar(ST[:, R_OH + sv, :], rem[:],
                                            float(sv), 1.0, ALU.is_equal,
                                            ALU.mult)
                aP16 = scr.tile([128, C], FP16, tag="aP16")
                nc.gpsimd.tensor_copy(aP16[:], amtP)
                qm = scr.tile([128, C], FP16, tag="qm")
                for qv in range(QP):
                    nc.vector.tensor_scalar(qm[:], q16[:], float(qv), 1.0,
                                            ALU.is_equal, ALU.mult)
                    nc.gpsimd.tensor_tensor(MV[:, M_WQ + qv, :], qm[:], aP16[:],
                                            ALU.mult)
                nc.scalar.activation(ST[:, R_TI, :], idsP, AFT.Tanh, scale=4.0,
                                     bias=0.0)
                nc.scalar.activation(MV[:, M_TA, :], amtP, AFT.Tanh, scale=4.0,
                                     bias=0.0)

                # ---- per-batch Grams ----
                ps = ppool.tile([NST, SL * NMV], FP32, tag="gram")
                for j in range(C):
                    bb = j // CH
                    cc = j % CH
                    nc.tensor.matmul(ps[:, bb * NMV:(bb + 1) * NMV],
                                     ST[:, :, j], MV[:, :, j],
                                     start=(cc == 0), stop=(cc == CH - 1))
                osb = opool.tile([NST, SL * NMV], FP32, tag="osb")
                nc.scalar.copy(osb[:], ps[:])
                nc.sync.dma_start(gout.ap()[:, s, :], osb[:])
    nc.compile()
    return nc


@functools.lru_cache(maxsize=2)
def _compiled():
    return _build()


def _fold_G_true(coeffs: np.ndarray) -> np.ndarray:
    """G[f, r]: sum_{f<FT, r<UT} G * T_f(T_30(x)) * (T_r(x) + s_r) == chebval.
    s_r = 1 for even r >= 2 (device stores those shifted).  Exact in fp64."""
    N = 450
    M = np.zeros((N, FT * UT))
    for f in range(FT):
        for r in range(UT):
            col = f * UT + r
            a = 30 * f
            M[a + r, col] += 0.5
            M[abs(a - r), col] += 0.5
            if r >= 2 and r % 2 == 0:
                M[a, col] += 1.0
    c = np.zeros(N)
    c[:len(coeffs)] = coeffs
    g, _, _, _ = np.linalg.lstsq(M, c, rcond=None)
    return g.reshape(FT, UT)


def _fold_G_pred(coeffs: np.ndarray) -> np.ndarray:
    q = np.arange(QP)[:, None]
    sv = np.arange(SP)[None, :]
    k = 28 * q + sv
    return Ch.chebval(np.minimum(k, 222) / 111.0 - 1.0, coeffs)


def kernel(y_pred: np.ndarray, y: np.ndarray, calories_coeffs: np.ndarray,
           _trace: bool = False):
    coeffs = np.asarray(calories_coeffs, np.float64)
    GT = _fold_G_true(coeffs)
    GP = _fold_G_pred(coeffs)

    ypf = np.ascontiguousarray(y_pred.reshape(B, J, 2), np.float32)
    ytf = np.ascontiguousarray(y.reshape(B, J, 2), np.float32)
    in_maps = []
    for i in range(N_CORES):
        in_maps.append({
            "yp": ypf[i * BC:(i + 1) * BC],
            "yt": ytf[i * BC:(i + 1) * BC],
        })
    nc = _compiled()
    res = run_bass_kernel_spmd(nc, in_maps, list(range(N_CORES)), trace=_trace)

    pens = np.zeros(B)
    diffs = np.zeros(B)
    rmap = [0] + list(range(1, UT))          # stationary row for U_r
    for ci, r in enumerate(res.results):
        g = np.asarray(r["gout"], np.float64)     # [NST, NSL, SL*NMV]
        for s in range(NSL):
            for bi in range(SL):
                Mb = g[:, s, bi * NMV:(bi + 1) * NMV]
                b = ci * BC + s * SL + bi
                calT = np.einsum("fr,rf->", GT, Mb[rmap, :FT])
                calP = np.einsum("qs,sq->", GP, Mb[R_OH:R_OH + SP, M_WQ:M_WQ + QP])
                diffs[b] = (calT - calP) / 700.0
                pens[b] = Mb[R_TI, M_ONE] + Mb[0, M_TA] - 2.0 * Mb[R_TI, M_TA]
    loss = pens.mean() + (diffs ** 2).mean()
    out = np.float32(loss)
    if _trace:
        return out, res
    return out
